# revision 2
# baseline (speedup 1.0000x reference)
"""DimensionalConsistencyLoss on 8 Trainium2 NeuronCores.

Strategy: the loss only touches gathered rows of the [100000, 512] embedding
table -- 4096 pos + 4096 neg constraints each read one full row (for the
sparsity term) plus the target element; 2048 neu constraints read a single
element. Constraints are partitioned by index across the 8 cores (each core
sees the full table in HBM, so no vocab-ownership bookkeeping or collectives
are needed). Per core: indirect-DMA gather of 1024 rows + 1280 target
elements, a fused abs-reduce per row tile, a handful of small elementwise ops
for the sign loss, and a ones-matmul partition reduce down to one scalar.
The 8 partial sums are combined and scaled on the host.
"""

import numpy as np

import concourse.bacc as bacc
import concourse.bass as bass
import concourse.mybir as mybir
import concourse.tile as tile
from concourse.bass_utils import run_bass_kernel_spmd

P = 128
VOCAB = 100000
DIM = 512
N_POS = 4096
N_NEG = 4096
N_NEU = 2048
N_CORES = 8

NPOS_C = N_POS // N_CORES   # 512
NNEG_C = N_NEG // N_CORES   # 512
NNEU_C = N_NEU // N_CORES   # 256
RT = (NPOS_C + NNEG_C) // P          # 8 row-gather tiles (cols 0-3 pos, 4-7 neg)
ET = (NPOS_C + NNEG_C + NNEU_C) // P  # 10 element cols (8-9 neu)
NPC = NPOS_C // P                     # 4 pos columns

CONSISTENCY_WEIGHT = 0.5
SPARSITY_WEIGHT = 0.1
C_SP = SPARSITY_WEIGHT / (DIM - 1)

F32 = mybir.dt.float32
I32 = mybir.dt.int32
AX = mybir.AxisListType.X
OP = mybir.AluOpType

_nc_cache = None


def _build_program():
    global _nc_cache
    if _nc_cache is not None:
        return _nc_cache

    nc = bacc.Bacc(
        "TRN2", target_bir_lowering=False, debug=False, num_devices=N_CORES
    )
    emb = nc.dram_tensor("emb", [VOCAB, DIM], F32, kind="ExternalInput")
    ridx_d = nc.dram_tensor("row_idx", [P, RT], I32, kind="ExternalInput")
    eidx_d = nc.dram_tensor("elem_idx", [P, ET], I32, kind="ExternalInput")
    out_d = nc.dram_tensor("out", [1, 1], F32, kind="ExternalOutput")

    with tile.TileContext(nc) as tc:
        with (
            tc.tile_pool(name="sb", bufs=1) as pool,
            tc.tile_pool(name="rowp", bufs=4) as rowpool,
            tc.tile_pool(name="ps", bufs=1, space="PSUM") as pp,
        ):
            ridx = pool.tile([P, RT], I32)
            nc.sync.dma_start(ridx[:], ridx_d[:, :])
            eidx = pool.tile([P, ET], I32)
            nc.sync.dma_start(eidx[:], eidx_d[:, :])

            # Gather target elements emb[id, dim] for all 1280 constraints.
            # Flat index id*DIM+dim, axis=1 -> coef 1 over the flat view.
            # HW honors ONE index per partition per indirect DMA, so one
            # [P,1] gather per column.
            tv = pool.tile([P, ET], F32)
            for k in range(ET):
                nc.gpsimd.indirect_dma_start(
                    out=tv[:, k : k + 1],
                    out_offset=None,
                    in_=emb[:, :],
                    in_offset=bass.IndirectOffsetOnAxis(ap=eidx[:, k : k + 1], axis=1),
                )

            # Gather full rows for pos+neg; fused abs-sum reduce per tile.
            rowsum = pool.tile([P, RT], F32)
            for t in range(RT):
                rows = rowpool.tile([P, DIM], F32, tag="rows")
                nc.gpsimd.indirect_dma_start(
                    out=rows[:],
                    out_offset=None,
                    in_=emb[:, :],
                    in_offset=bass.IndirectOffsetOnAxis(ap=ridx[:, t : t + 1], axis=0),
                )
                nc.vector.reduce_sum(
                    rowsum[:, t : t + 1], rows[:], axis=AX, apply_absolute_value=True
                )

            # w = wrong-sign indicator: pos t<=0, neg t>=0; neu w=(t>=0) (only
            # used to rebuild |t|).
            w = pool.tile([P, ET], F32)
            nc.vector.tensor_scalar(
                out=w[:, 0:NPC], in0=tv[:, 0:NPC], scalar1=0.0, scalar2=None,
                op0=OP.is_le,
            )
            nc.vector.tensor_scalar(
                out=w[:, NPC:ET], in0=tv[:, NPC:ET], scalar1=0.0, scalar2=None,
                op0=OP.is_ge,
            )
            # m maps t -> |t|: pos 1-2w, others 2w-1
            m = pool.tile([P, ET], F32)
            nc.vector.tensor_scalar(
                out=m[:, 0:NPC], in0=w[:, 0:NPC], scalar1=-2.0, scalar2=1.0,
                op0=OP.mult, op1=OP.add,
            )
            nc.vector.tensor_scalar(
                out=m[:, NPC:ET], in0=w[:, NPC:ET], scalar1=2.0, scalar2=-1.0,
                op0=OP.mult, op1=OP.add,
            )
            a = pool.tile([P, ET], F32)
            nc.vector.tensor_tensor(out=a[:], in0=tv[:], in1=m[:], op=OP.mult)

            # sign loss = -0.1a + w*(1.1a + 0.1); sparsity = c*(rowsum - a).
            # Fold: L = (-0.1 - c)*a + w*(1.1a + 0.1) + c*rowsum   (pos/neg)
            #       L = 2a                                          (neu)
            x1 = pool.tile([P, RT], F32)
            nc.vector.tensor_scalar(
                out=x1[:], in0=a[:, 0:RT], scalar1=1.1, scalar2=0.1,
                op0=OP.mult, op1=OP.add,
            )
            L = pool.tile([P, ET], F32)
            nc.vector.tensor_tensor(out=L[:, 0:RT], in0=w[:, 0:RT], in1=x1[:], op=OP.mult)
            x2 = pool.tile([P, RT], F32)
            nc.vector.tensor_scalar(
                out=x2[:], in0=a[:, 0:RT], scalar1=(-0.1 - C_SP), scalar2=None,
                op0=OP.mult,
            )
            nc.vector.tensor_tensor(out=L[:, 0:RT], in0=L[:, 0:RT], in1=x2[:], op=OP.add)
            x3 = pool.tile([P, RT], F32)
            nc.vector.tensor_scalar(
                out=x3[:], in0=rowsum[:], scalar1=C_SP, scalar2=None, op0=OP.mult
            )
            nc.vector.tensor_tensor(out=L[:, 0:RT], in0=L[:, 0:RT], in1=x3[:], op=OP.add)
            nc.vector.tensor_scalar(
                out=L[:, RT:ET], in0=a[:, RT:ET], scalar1=2.0, scalar2=None,
                op0=OP.mult,
            )

            # Reduce [P, ET] -> [P, 1] -> scalar via ones-matmul.
            Lc = pool.tile([P, 1], F32)
            nc.vector.reduce_sum(Lc[:], L[:], axis=AX)
            ones = pool.tile([P, 1], F32)
            nc.vector.memset(ones[:], 1.0)
            acc = pp.tile([1, 1], F32)
            nc.tensor.matmul(acc[:], lhsT=ones[:], rhs=Lc[:], start=True, stop=True)
            res = pool.tile([1, 1], F32)
            nc.vector.tensor_copy(res[:], acc[:])
            nc.sync.dma_start(out_d[:, :], res[:])

    nc.compile()
    _nc_cache = nc
    return nc


def _make_in_maps(emb, pos_ids, pos_dims, neg_ids, neg_dims, neu_ids, neu_dims):
    in_maps = []
    for c in range(N_CORES):
        pid = pos_ids[c * NPOS_C : (c + 1) * NPOS_C]
        pdim = pos_dims[c * NPOS_C : (c + 1) * NPOS_C]
        nid = neg_ids[c * NNEG_C : (c + 1) * NNEG_C]
        ndim = neg_dims[c * NNEG_C : (c + 1) * NNEG_C]
        uid = neu_ids[c * NNEU_C : (c + 1) * NNEU_C]
        udim = neu_dims[c * NNEU_C : (c + 1) * NNEU_C]

        row_ids = np.concatenate([pid, nid])  # [1024]
        row_idx = np.ascontiguousarray(
            row_ids.reshape(RT, P).T.astype(np.int32)
        )  # [128, 8]; col j = constraints j*128:(j+1)*128
        flat = np.concatenate(
            [pid * DIM + pdim, nid * DIM + ndim, uid * DIM + udim]
        )  # [1280]
        elem_idx = np.ascontiguousarray(flat.reshape(ET, P).T.astype(np.int32))

        in_maps.append({"emb": emb, "row_idx": row_idx, "elem_idx": elem_idx})
    return in_maps


def kernel(**inputs):
    emb = np.ascontiguousarray(np.asarray(inputs["embeddings"], dtype=np.float32))
    ids = {
        k: np.asarray(inputs[k]).astype(np.int64)
        for k in ("pos_ids", "pos_dims", "neg_ids", "neg_dims", "neu_ids", "neu_dims")
    }
    nc = _build_program()
    in_maps = _make_in_maps(
        emb, ids["pos_ids"], ids["pos_dims"], ids["neg_ids"], ids["neg_dims"],
        ids["neu_ids"], ids["neu_dims"],
    )
    res = run_bass_kernel_spmd(nc, in_maps, list(range(N_CORES)))
    total = sum(float(r["out"][0, 0]) for r in res.results)
    val = total * CONSISTENCY_WEIGHT / (N_POS + N_NEG + N_NEU)
    return np.asarray(val, dtype=np.float32)
